# revision 3
# baseline (speedup 1.0000x reference)
"""Trainium2 Bass kernel for nn_DSQGAttentionQW (sparse offset attention).

Sharding: head-tensor-parallel attention (8 heads -> 8 cores) + AllToAll
re-shard to sequence-parallel for the output projection. Single NEFF launch.
"""
import math
import numpy as np

import concourse.bacc as bacc
import concourse.bass as bass
import concourse.tile as tile
import concourse.mybir as mybir
import concourse.masks as masks
from concourse.bass_utils import run_bass_kernel_spmd

# ---- problem constants (must match reference.py) ----
_DENSE_LOCAL_W = 32
_DYADIC = [48, 64, 96, 128, 192, 256, 384, 512, 768, 1024, 1536, 2048, 3072, 4096]
OFFSETS = np.array(
    sorted(set(range(0, _DENSE_LOCAL_W + 1)) | set(_DYADIC)), dtype=np.int32
)  # [47]
NUM_OFFSETS = len(OFFSETS)
H = 8
_LOG_MAX = math.log(1.0 + 4096.0)
_HEAD_OMEGAS = [0.0, 0.0, 1 * math.pi / _LOG_MAX, 1 * math.pi / _LOG_MAX,
                4 * math.pi / _LOG_MAX, 4 * math.pi / _LOG_MAX,
                6 * math.pi / _LOG_MAX, 6 * math.pi / _LOG_MAX]
_log_d = np.log(1.0 + OFFSETS.astype(np.float64))
DISP_COS_KERNEL = np.zeros((NUM_OFFSETS, H), dtype=np.float32)
for _h, _om in enumerate(_HEAD_OMEGAS):
    if _om > 0.0:
        DISP_COS_KERNEL[:, _h] = np.cos(_om * _log_d)

B, N, D = 1, 2048, 512
HD = D // H
NC = 8
NB = N // NC            # 256: per-core output row block
NT = N // 128           # 16 q-tiles of 128
# k-tile depths m such that some offset delta maps into k-tile (t - m)
R_DEPTHS = [0, 1, 2, 3, 4, 6, 8, 12, 16, 24, 32]
NR = len(R_DEPTHS)

FP = mybir.dt.float32

_cache = {}


def _build_masks(eff_pb_h: np.ndarray) -> np.ndarray:
    """maskW[ri, kp, i] = exp(eff_pb[offset_idx(delta)]) if delta valid else 0,
    with delta = i - kp + 128*m for depth m = R_DEPTHS[ri]."""
    off_idx = {int(d): i for i, d in enumerate(OFFSETS)}
    kp = np.arange(128)[None, :, None]
    i = np.arange(128)[None, None, :]
    m = np.array(R_DEPTHS)[:, None, None]
    delta = i - kp + 128 * m  # [NR, 128, 128]
    w = np.zeros((NR, 128, 128), dtype=np.float32)
    for d, oi in off_idx.items():
        w[delta == d] = math.exp(float(eff_pb_h[oi]))
    return w


def _build_module():
    nc = bacc.Bacc("TRN2", target_bir_lowering=False, debug=False, num_devices=NC)

    xT = nc.dram_tensor("xT", [D, N], FP, kind="ExternalInput").ap()
    wA = nc.dram_tensor("wA", [D, 128], FP, kind="ExternalInput").ap()   # [Wq|Wk]
    wB = nc.dram_tensor("wB", [D, 128], FP, kind="ExternalInput").ap()   # [Wv|Wg]
    bA = nc.dram_tensor("bA", [128], FP, kind="ExternalInput").ap()
    bB = nc.dram_tensor("bB", [128], FP, kind="ExternalInput").ap()
    maskW = nc.dram_tensor("maskW", [NR, 128, 128], FP, kind="ExternalInput").ap()
    woutS = nc.dram_tensor("woutS", [HD, H, D], FP, kind="ExternalInput").ap()
    bout = nc.dram_tensor("bout", [D], FP, kind="ExternalInput").ap()
    yout = nc.dram_tensor("y", [NB, D], FP, kind="ExternalOutput").ap()

    with tile.TileContext(nc) as tc:
        with (
            tc.tile_pool(name="singles", bufs=1) as S,
            tc.tile_pool(name="work", bufs=3) as W,
            tc.tile_pool(name="ptiles", bufs=2) as PT,
            tc.tile_pool(name="ps", bufs=2, space="PSUM") as PS,
            tc.tile_pool(name="dram", bufs=1, space="DRAM") as DR,
        ):
            # ---------- constants / loads ----------
            ident = S.tile([128, 128], FP)
            masks.make_identity(nc, ident[:])
            ones_r = S.tile([1, 128], FP)
            nc.vector.memset(ones_r[:], 1.0)

            xs = S.tile([128, 4, N], FP)
            xT_r = xT.rearrange("(ct p) n -> p ct n", p=128)
            for ct in range(4):
                nc.sync.dma_start(out=xs[:, ct, :], in_=xT_r[:, ct, :])

            wAs = S.tile([128, 4, 128], FP)
            nc.sync.dma_start(out=wAs[:], in_=wA.rearrange("(ct p) o -> p ct o", p=128))
            wBs = S.tile([128, 4, 128], FP)
            nc.sync.dma_start(out=wBs[:], in_=wB.rearrange("(ct p) o -> p ct o", p=128))
            bAs = S.tile([128, 1], FP)
            nc.sync.dma_start(out=bAs[:], in_=bA[:, None])
            bBs = S.tile([128, 1], FP)
            nc.sync.dma_start(out=bBs[:], in_=bB[:, None])
            mws = S.tile([128, NR, 128], FP)
            nc.sync.dma_start(out=mws[:], in_=maskW.rearrange("r kp i -> kp r i"))
            wos = S.tile([HD, H, D], FP)
            nc.sync.dma_start(out=wos[:], in_=woutS[:])
            bos = S.tile([1, D], FP)
            nc.sync.dma_start(out=bos[:], in_=bout[None, :])

            # ---------- MM-A: qT / kT / vT / gT ----------
            qT = S.tile([64, N], FP)       # pre-scaled by 1/sqrt(HD)
            kT = S.tile([64, N], FP)
            vT = S.tile([64, N], FP)       # if_gain folded
            gT = S.tile([64, N], FP)       # sigmoid gate
            for nch in range(4):
                nsl = slice(512 * nch, 512 * (nch + 1))
                psA = PS.tile([128, 512], FP, tag="mma")
                psB = PS.tile([128, 512], FP, tag="mma")
                for ct in range(4):
                    nc.tensor.matmul(psA[:], wAs[:, ct, :], xs[:, ct, nsl],
                                     start=(ct == 0), stop=(ct == 3))
                for ct in range(4):
                    nc.tensor.matmul(psB[:], wBs[:, ct, :], xs[:, ct, nsl],
                                     start=(ct == 0), stop=(ct == 3))
                nc.scalar.activation(qT[:, nsl], psA[0:64, :],
                                     mybir.ActivationFunctionType.Identity,
                                     bias=bAs[0:64], scale=1.0)
                nc.scalar.activation(kT[:, nsl], psA[64:128, :],
                                     mybir.ActivationFunctionType.Identity,
                                     bias=bAs[64:128], scale=1.0)
                nc.scalar.activation(vT[:, nsl], psB[0:64, :],
                                     mybir.ActivationFunctionType.Identity,
                                     bias=bBs[0:64], scale=1.0)
                nc.scalar.activation(gT[:, nsl], psB[64:128, :],
                                     mybir.ActivationFunctionType.Sigmoid,
                                     bias=bBs[64:128], scale=1.0)

            # ---------- V natural + ones column ----------
            Vn = S.tile([128, NT, HD + 1], FP)
            nc.vector.memset(Vn[:, :, HD:HD + 1], 1.0)
            for t in range(NT):
                psT = PS.tile([128, 64], FP, tag="s")
                nc.tensor.transpose(psT[:], vT[:, 128 * t:128 * (t + 1)],
                                    ident[0:64, 0:64])
                nc.scalar.copy(Vn[:, t, 0:HD], psT[:])

            # ---------- attention per q-tile ----------
            zX = S.tile([HD + 1, N], FP)   # rows 0:64 gated attn out^T, row 64 denom
            for t in range(NT):
                Pt = PT.tile([128, NR, 128], FP, tag="P")
                for ri, m in enumerate(R_DEPTHS):
                    if t - m < 0:
                        nc.vector.memset(Pt[:, ri, :], 0.0)
                        continue
                    ksl = slice(128 * (t - m), 128 * (t - m + 1))
                    qsl = slice(128 * t, 128 * (t + 1))
                    psS = PS.tile([128, 128], FP, tag="s")
                    nc.tensor.matmul(psS[:], kT[:, ksl], qT[:, qsl],
                                     start=True, stop=True)
                    expS = W.tile([128, 128], FP, tag="expS")
                    nc.scalar.activation(expS[:], psS[:],
                                         mybir.ActivationFunctionType.Exp)
                    nc.vector.tensor_mul(Pt[:, ri, :], expS[:], mws[:, ri, :])
                valid = [(ri, m) for ri, m in enumerate(R_DEPTHS) if t - m >= 0]
                ps2 = PS.tile([HD + 1, 128], FP, tag="o2")
                for j, (ri, m) in enumerate(valid):
                    nc.tensor.matmul(ps2[:], Vn[:, t - m, :], Pt[:, ri, :],
                                     start=(j == 0), stop=(j == len(valid) - 1))
                qsl = slice(128 * t, 128 * (t + 1))
                nc.vector.tensor_mul(zX[0:HD, qsl], ps2[0:HD, :], gT[:, qsl])
                nc.scalar.copy(zX[HD:HD + 1, qsl], ps2[HD:HD + 1, :])

            # ---------- AllToAll exchange ----------
            bin_ = DR.tile([NC, HD + 1, NB], FP)
            bout_ = DR.tile([NC, HD + 1, NB], FP)
            for j in range(NC):
                nc.sync.dma_start(out=bin_[j], in_=zX[:, NB * j:NB * (j + 1)])
            nc.gpsimd.collective_compute(
                "AllToAll", mybir.AluOpType.bypass,
                replica_groups=[list(range(NC))],
                ins=[bin_[:].opt()], outs=[bout_[:].opt()],
            )

            # ---------- stage 3: normalize + out projection ----------
            zr = S.tile([HD, NC, NB], FP)
            for src in range(NC):
                nc.sync.dma_start(out=zr[:, src, :], in_=bout_[src, 0:HD, :])
            dens = S.tile([NC, NB], FP)
            nc.sync.dma_start(out=dens[:], in_=bout_[:, HD, :])
            recs = S.tile([NC, NB], FP)
            nc.vector.reciprocal(recs[:], dens[:])
            rdr = DR.tile([NC, NB], FP)
            nc.sync.dma_start(out=rdr[:], in_=recs[:])
            rb = S.tile([HD, NC, NB], FP)
            rdr_ap = rdr[:]
            rb_src = bass.AP(tensor=rdr_ap.tensor, offset=rdr_ap.offset,
                             ap=[[0, HD], *rdr_ap.ap])
            nc.sync.dma_start(out=rb[:], in_=rb_src)
            zn = S.tile([HD, NC, NB], FP)
            nc.vector.tensor_mul(zn[:], zr[:], rb[:])

            for nt in range(NB // 128):
                nsl = slice(128 * nt, 128 * (nt + 1))
                psY = PS.tile([128, D], FP, tag="y")
                for h in range(H):
                    nc.tensor.matmul(psY[:], zn[:, h, nsl], wos[:, h, :],
                                     start=(h == 0), stop=False)
                nc.tensor.matmul(psY[:], ones_r[:, 0:128], bos[:],
                                 start=False, stop=True)
                ysb = W.tile([128, D], FP, tag="ysb")
                nc.scalar.copy(ysb[:], psY[:])
                nc.sync.dma_start(out=yout[nsl, :], in_=ysb[:])

    nc.compile()
    return nc


def _prep_inputs(x, W_qkv, b_qkv, W_out, b_out, W_gate, b_gate,
                 pos_bias, scale_embed, if_gain, disp_amp):
    assert not np.any(np.asarray(scale_embed)), \
        "kernel fast path requires scale_embed == 0"
    xTn = np.ascontiguousarray(np.asarray(x)[0].T.astype(np.float32))  # [D, N]
    W_qkv = np.asarray(W_qkv, dtype=np.float32)
    b_qkv = np.asarray(b_qkv, dtype=np.float32)
    W_gate = np.asarray(W_gate, dtype=np.float32)
    b_gate = np.asarray(b_gate, dtype=np.float32)
    W_out = np.asarray(W_out, dtype=np.float32)
    b_out = np.asarray(b_out, dtype=np.float32)
    pos_bias = np.asarray(pos_bias, dtype=np.float32)
    if_gain = np.asarray(if_gain, dtype=np.float32)
    disp_amp = np.asarray(disp_amp, dtype=np.float32)

    scl = 1.0 / math.sqrt(HD)
    woutS = np.ascontiguousarray(
        W_out.reshape(H, HD, D).transpose(1, 0, 2))  # [HD, H, D]

    in_maps = []
    for h in range(NC):
        qs = slice(HD * h, HD * (h + 1))
        ks = slice(D + HD * h, D + HD * (h + 1))
        vs = slice(2 * D + HD * h, 2 * D + HD * (h + 1))
        wq = W_qkv[:, qs] * scl
        wk = W_qkv[:, ks]
        wv = W_qkv[:, vs] * if_gain[h]
        wg = W_gate[:, qs]
        bq = b_qkv[qs] * scl
        bk = b_qkv[ks]
        bv = b_qkv[vs] * if_gain[h]
        bg = b_gate[qs]
        eff_pb_h = pos_bias[:, h] + DISP_COS_KERNEL[:, h] * disp_amp[h]
        in_maps.append({
            "xT": xTn,
            "wA": np.ascontiguousarray(np.concatenate([wq, wk], axis=1)),
            "wB": np.ascontiguousarray(np.concatenate([wv, wg], axis=1)),
            "bA": np.ascontiguousarray(np.concatenate([bq, bk])),
            "bB": np.ascontiguousarray(np.concatenate([bv, bg])),
            "maskW": _build_masks(eff_pb_h),
            "woutS": woutS,
            "bout": b_out,
        })
    return in_maps


def kernel(**inputs) -> np.ndarray:
    if "nc" not in _cache:
        _cache["nc"] = _build_module()
    nc = _cache["nc"]
    in_maps = _prep_inputs(**inputs)
    res = run_bass_kernel_spmd(nc, in_maps, core_ids=list(range(NC)))
    y = np.concatenate([res.results[c]["y"] for c in range(NC)], axis=0)
    return y.reshape(B, N, D)
